# revision 35
# baseline (speedup 1.0000x reference)
"""TRN2 Bass kernel for nn_AE_14413910245386 (autoencoder + cosine-topk loss).

Data-parallel over 8 NeuronCores: x [131072,224] sharded along batch into
8 x [16384,224]; MLP weights + data_input replicated. Each core computes its
y/z shard plus per-partition top-20 cosine candidates and a partial MSE sum;
the host merges candidates (local top-20 -> global top-20) and partial sums
into R_loss during the unshard step.

Per-core layout: feature-major activations (hT [feat, batch]) so the PE
contracts over the partition dim; batch tiles of 512. x is transposed on-chip
via the DMA xbar (bf16); the last layer runs batch-major (lhsT = g2T chunk)
so y lands batch-major in PSUM and needs no transpose back. bd3 is folded
into the last matmul via a ones-row in lhsT / bias-row in rhs.
"""
import numpy as np

import concourse.bacc as bacc
import concourse.tile as tile
import concourse.mybir as mybir
import concourse.bass_utils as bass_utils

N, D, H, Z = 131072, 224, 400, 4
TOPK = 20
NCORES = 8
NLOC = N // NCORES          # 16384 rows per core
BT = 512                    # batch tile
NT = NLOC // BT             # 32 tiles
HCH = [(0, 128), (128, 128), (256, 128), (384, 16)]   # H=400 chunking
DCH = [(0, 128), (128, 96)]                            # D=224 chunking
NCOL = NT * (BT // 128)     # 128 accumulator columns (one per 128-sample chunk)

FP32 = mybir.dt.float32
BF16 = mybir.dt.bfloat16
AF = mybir.ActivationFunctionType
ALU = mybir.AluOpType
AX = mybir.AxisListType

_cached = None


def _build():
    nc = bacc.Bacc("TRN2", target_bir_lowering=False, debug=False,
                   num_devices=NCORES)

    x_d = nc.dram_tensor("x", [NLOC, D], FP32, kind="ExternalInput")
    di_d = nc.dram_tensor("data_input", [D, 1], FP32, kind="ExternalInput")
    w_d = {}
    for name, shape in [("We1", [D, H]), ("be1", [H]), ("We2", [H, H]),
                        ("be2", [H]), ("We3", [H, Z]), ("be3", [Z]),
                        ("Wd1", [Z, H]), ("bd1", [H]), ("Wd2", [H, H]),
                        ("bd2", [H]), ("Wd3", [H, D]), ("bd3", [D])]:
        w_d[name] = nc.dram_tensor(name, shape, FP32, kind="ExternalInput")

    y_d = nc.dram_tensor("y", [NLOC, D], FP32, kind="ExternalOutput")
    # z stored feature-major [Z, NLOC] (contiguous DMA); host transposes back
    z_d = nc.dram_tensor("z", [Z, NLOC], FP32, kind="ExternalOutput")
    cand_d = nc.dram_tensor("cand", [128, TOPK], FP32, kind="ExternalOutput")
    msep_d = nc.dram_tensor("msep", [128, 1], FP32, kind="ExternalOutput")

    with tile.TileContext(nc) as tc:
        with (
            tc.tile_pool(name="wp", bufs=1) as wp,
            tc.tile_pool(name="stg", bufs=2) as stg,
            tc.tile_pool(name="persist", bufs=1) as pp,
            tc.tile_pool(name="xbm", bufs=4) as xbm_p,
            tc.tile_pool(name="xbf", bufs=3) as xbf_p,
            tc.tile_pool(name="xT", bufs=4) as xT_p,
            tc.tile_pool(name="act", bufs=2) as act_p,
            tc.tile_pool(name="ybm", bufs=3) as ybm_p,
            tc.tile_pool(name="scr", bufs=3) as scr_p,
            tc.tile_pool(name="ps", bufs=8, space="PSUM") as ps_p,
        ):
            # ---------------- weight prep (once) ----------------
            def load_lhsT(wname, kchunks, mchunks):
                """W [K, M] -> bf16 lhsT tiles per (kchunk, mchunk)."""
                tiles = {}
                for ki, (k0, kk) in enumerate(kchunks):
                    for mi, (m0, mm) in enumerate(mchunks):
                        s = stg.tile([kk, mm], FP32, tag="wstg")
                        nc.sync.dma_start(s[:], w_d[wname].ap()[k0:k0 + kk, m0:m0 + mm])
                        t = wp.tile([kk, mm], BF16, tag=f"{wname}_{ki}_{mi}")
                        nc.vector.tensor_copy(t[:], s[:])
                        tiles[(ki, mi)] = t
                return tiles

            We1_t = load_lhsT("We1", DCH, HCH)
            We2_t = load_lhsT("We2", HCH, HCH)
            We3_t = load_lhsT("We3", HCH, [(0, Z)])
            Wd1_t = load_lhsT("Wd1", [(0, Z)], HCH)
            Wd2_t = load_lhsT("Wd2", HCH, HCH)

            # Wd3 as rhs tiles [kk, D]; bd3 as its own rhs row (added to the
            # L6 accumulation via a K=1 matmul against a ones-row lhsT)
            Wd3_t = []
            for ki, (k0, kk) in enumerate(HCH):
                s = stg.tile([kk, D], FP32, tag="wstg_d3")
                nc.sync.dma_start(s[:], w_d["Wd3"].ap()[k0:k0 + kk, :])
                t = wp.tile([kk, D], BF16, tag=f"Wd3_{ki}")
                nc.vector.tensor_copy(t[:], s[:])
                Wd3_t.append(t)
            sbd3 = stg.tile([1, D], FP32, tag="wstg_d3b")
            nc.sync.dma_start(sbd3[:], w_d["bd3"].ap())
            bd3_row = wp.tile([1, D], BF16, tag="bd3_row")
            nc.vector.tensor_copy(bd3_row[:], sbd3[:])
            ones_row = wp.tile([1, 128], BF16, tag="ones_row")
            nc.vector.memset(ones_row[:], 1.0)

            # identity [128,128] bf16 for PE-mode transposes
            iph = wp.tile([128, 128], mybir.dt.int32, tag="iph")
            nc.gpsimd.iota(iph[:], pattern=[[1, 128]], base=0,
                           channel_multiplier=0)
            iphf = wp.tile([128, 128], FP32, tag="iphf")
            nc.vector.tensor_copy(iphf[:], iph[:])
            ipp = wp.tile([128, 1], mybir.dt.int32, tag="ipp")
            nc.gpsimd.iota(ipp[:], pattern=[[0, 1]], base=0,
                           channel_multiplier=1)
            ippf = wp.tile([128, 1], FP32, tag="ippf")
            nc.vector.tensor_copy(ippf[:], ipp[:])
            ident = wp.tile([128, 128], BF16, tag="ident")
            nc.vector.tensor_scalar(out=ident[:], in0=iphf[:],
                                    scalar1=ippf[:, 0:1], scalar2=None,
                                    op0=ALU.is_equal)

            def load_bias(bname, chunks):
                out = []
                for mi, (m0, mm) in enumerate(chunks):
                    t = wp.tile([mm, 1], FP32, tag=f"{bname}_{mi}")
                    nc.sync.dma_start(t[:], w_d[bname].ap()[m0:m0 + mm])
                    out.append(t)
                return out

            be1_c = load_bias("be1", HCH)
            be2_c = load_bias("be2", HCH)
            be3_c = load_bias("be3", [(0, Z)])[0]
            bd1_c = load_bias("bd1", HCH)
            bd2_c = load_bias("bd2", HCH)

            # data_input -> dT row, broadcast [128, D]; C = ||d||
            dT = pp.tile([1, D], FP32, tag="dT")
            nc.sync.dma_start(dT[:], di_d.ap().rearrange("d one -> one d"))
            d_bc = pp.tile([128, D], FP32, tag="d_bc")
            nc.gpsimd.partition_broadcast(d_bc[:], dT[:])
            scrd = pp.tile([1, D], FP32, tag="scrd")
            C2 = pp.tile([1, 1], FP32, tag="C2")
            nc.vector.scalar_tensor_tensor(out=scrd[:], in0=dT[:], scalar=1.0,
                                           in1=dT[:], op0=ALU.mult, op1=ALU.mult,
                                           accum_out=C2[:])
            Cv = pp.tile([1, 1], FP32, tag="Cv")
            nc.scalar.activation(Cv[:], C2[:], AF.Sqrt)
            C_bc = pp.tile([128, 1], FP32, tag="C_bc")
            nc.gpsimd.partition_broadcast(C_bc[:], Cv[:])

            # accumulators
            Abuf = pp.tile([128, NCOL], FP32, tag="Abuf")
            B2buf = pp.tile([128, NCOL], FP32, tag="B2buf")
            T2buf = pp.tile([128, NCOL], FP32, tag="T2buf")

            # ------------- main loop: x-prep prefetched 2 tiles ahead ------
            SUP = 2
            loaded = {}
            prep = {}

            def do_load(t):
                s0 = t * BT
                xc, xbf = [], []
                for c in range(4):
                    r0 = s0 + 128 * c
                    xt = xbm_p.tile([128, D], FP32, tag=f"xbm{c}")
                    nc.gpsimd.dma_start(xt[:], x_d.ap()[r0:r0 + 128, :])
                    xc.append(xt)
                    xb = xbf_p.tile([128, 256], BF16, tag=f"xbf{c}")
                    nc.vector.tensor_copy(xb[:, 0:D], xt[:])
                    xbf.append(xb)
                loaded[t] = (xc, xbf)

            def do_transpose(t):
                # DMA-xbar transposes on the Sync queue (prefetched two tiles
                # ahead, off the PE critical path). xT1 rows 96:128 come from
                # the uninitialized xbf columns 224:256 and are never read.
                xc, xbf = loaded.pop(t)
                xT0 = xT_p.tile([128, BT], BF16, tag="xT0")
                xT1 = xT_p.tile([128, BT], BF16, tag="xT1")
                for c in range(4):
                    nc.sync.dma_start(xT0[:, 128 * c:128 * (c + 1)],
                                      xbf[c][:, 0:128], transpose=True)
                    nc.sync.dma_start(xT1[:, 128 * c:128 * (c + 1)],
                                      xbf[c][:, 128:256], transpose=True)
                prep[t] = (xc, [xT0, xT1])

            for tp in range(min(SUP, NT)):
                do_load(tp)

            def L1(xTs):
                pss = [ps_p.tile([mm, BT], FP32, tag="ps", name=f"ps_h1_{mi}")
                       for mi, (m0, mm) in enumerate(HCH)]
                for ki, (k0, kk) in enumerate(DCH):
                    for mi in range(4):
                        nc.tensor.matmul(pss[mi][:], We1_t[(ki, mi)][:],
                                         xTs[ki][0:kk, :],
                                         start=(ki == 0), stop=(ki == 1))
                h1 = []
                for mi, (m0, mm) in enumerate(HCH):
                    ht = act_p.tile([mm, BT], BF16, tag=f"h1_{mi}")
                    nc.scalar.activation(ht[:], pss[mi][:], AF.Prelu,
                                         bias=be1_c[mi][:], alpha=0.1)
                    h1.append(ht)
                return h1

            def LMID(h_in, W_t, b_c, name):
                # K-outer emission: all 4 PSUM groups allocated up front,
                # K-chunks interleaved -> slot-wait stalls concentrate once
                pss = [ps_p.tile([mm, BT], FP32, tag="ps", name=f"ps_{name}_{mi}")
                       for mi, (m0, mm) in enumerate(HCH)]
                for ki in range(4):
                    for mi in range(4):
                        nc.tensor.matmul(pss[mi][:], W_t[(ki, mi)][:],
                                         h_in[ki][:],
                                         start=(ki == 0), stop=(ki == 3))
                out = []
                for mi, (m0, mm) in enumerate(HCH):
                    ht = act_p.tile([mm, BT], BF16, tag=f"{name}_{mi}")
                    nc.scalar.activation(ht[:], pss[mi][:], AF.Prelu,
                                         bias=b_c[mi][:], alpha=0.1)
                    out.append(ht)
                return out

            def L3(h2, s0):
                ps = ps_p.tile([Z, BT], FP32, tag="ps")
                for ki in range(4):
                    nc.tensor.matmul(ps[:], We3_t[(ki, 0)][:], h2[ki][:],
                                     start=(ki == 0), stop=(ki == 3))
                zbf = act_p.tile([Z, BT], BF16, tag="zbf")
                nc.scalar.activation(zbf[:], ps[:], AF.Identity,
                                     bias=be3_c[:])
                z_f = act_p.tile([Z, BT], FP32, tag="z_f")
                nc.vector.tensor_scalar(out=z_f[:], in0=ps[:],
                                        scalar1=be3_c[:], scalar2=None,
                                        op0=ALU.add)
                nc.sync.dma_start(z_d.ap()[:, s0:s0 + BT], z_f[:])
                return zbf

            def L4(zbf):
                g1 = []
                for mi, (m0, mm) in enumerate(HCH):
                    ps = ps_p.tile([mm, BT], FP32, tag="ps")
                    nc.tensor.matmul(ps[:], Wd1_t[(0, mi)][:], zbf[:],
                                     start=True, stop=True)
                    ht = act_p.tile([mm, BT], BF16, tag=f"g1_{mi}")
                    nc.scalar.activation(ht[:], ps[:], AF.Prelu,
                                         bias=bd1_c[mi][:], alpha=0.1)
                    g1.append(ht)
                return g1

            def L6(g2, xc, t, s0):
                pss = [ps_p.tile([128, D], FP32, tag="ps", name=f"ps_y_{s}")
                       for s in range(4)]
                for ki in range(4):
                    for s in range(4):
                        nc.tensor.matmul(pss[s][:],
                                         g2[ki][:, 128 * s:128 * (s + 1)],
                                         Wd3_t[ki][:],
                                         start=(ki == 0), stop=False)
                for s in range(4):
                    nc.tensor.matmul(pss[s][:], ones_row[:], bd3_row[:],
                                     start=False, stop=True)
                for s in range(4):
                    ps = pss[s]
                    yt = ybm_p.tile([128, D], FP32, tag=f"ybm{s}")
                    nc.scalar.activation(yt[:], ps[:], AF.Tanh)
                    nc.sync.dma_start(y_d.ap()[s0 + 128 * s:s0 + 128 * (s + 1), :],
                                      yt[:])
                    # fused row-reductions into accumulator column j
                    j = 4 * t + s
                    sA = scr_p.tile([128, D], FP32, tag="sA")
                    nc.vector.scalar_tensor_tensor(
                        out=sA[:], in0=yt[:], scalar=1.0, in1=d_bc[:],
                        op0=ALU.mult, op1=ALU.mult,
                        accum_out=Abuf[:, j:j + 1])
                    sB = scr_p.tile([128, D], FP32, tag="sB")
                    nc.vector.scalar_tensor_tensor(
                        out=sB[:], in0=yt[:], scalar=1.0, in1=yt[:],
                        op0=ALU.mult, op1=ALU.mult,
                        accum_out=B2buf[:, j:j + 1])
                    td = scr_p.tile([128, D], FP32, tag="td")
                    nc.vector.tensor_tensor(out=td[:], in0=yt[:], in1=xc[s][:],
                                            op=ALU.subtract)
                    sC = scr_p.tile([128, D], FP32, tag="sC")
                    nc.vector.scalar_tensor_tensor(
                        out=sC[:], in0=td[:], scalar=1.0, in1=td[:],
                        op0=ALU.mult, op1=ALU.mult,
                        accum_out=T2buf[:, j:j + 1])

            do_transpose(0)
            do_transpose(1)
            for t0 in range(0, NT, 2):
                t1 = t0 + 1
                (xc0, xTs0), (xc1, xTs1) = prep.pop(t0), prep.pop(t1)
                h1_0 = L1(xTs0)
                h1_1 = L1(xTs1)
                h2_0 = LMID(h1_0, We2_t, be2_c, "h2")
                h2_1 = LMID(h1_1, We2_t, be2_c, "h2")
                zbf0 = L3(h2_0, t0 * BT)
                zbf1 = L3(h2_1, t1 * BT)
                if t0 + 2 < NT:
                    do_load(t0 + 2)
                    do_transpose(t0 + 2)
                if t1 + 2 < NT:
                    do_load(t1 + 2)
                    do_transpose(t1 + 2)
                g1_0 = L4(zbf0)
                g1_1 = L4(zbf1)
                g2_0 = LMID(g1_0, Wd2_t, bd2_c, "g2")
                g2_1 = LMID(g1_1, Wd2_t, bd2_c, "g2")
                L6(g2_0, xc0, t0, t0 * BT)
                L6(g2_1, xc1, t1, t1 * BT)

            # ---------------- end phase ----------------
            msep_sb = pp.tile([128, 1], FP32, tag="msep_sb")
            nc.vector.reduce_sum(msep_sb[:], T2buf[:], axis=AX.X)
            nc.sync.dma_start(msep_d.ap(), msep_sb[:])

            Bsq = pp.tile([128, NCOL], FP32, tag="Bsq")
            nc.scalar.activation(Bsq[:], B2buf[:], AF.Sqrt)
            den = pp.tile([128, NCOL], FP32, tag="den")
            nc.vector.tensor_scalar(out=den[:], in0=Bsq[:], scalar1=C_bc[:, 0:1],
                                    scalar2=1e-5, op0=ALU.mult, op1=ALU.add)
            rec = pp.tile([128, NCOL], FP32, tag="rec")
            nc.vector.reciprocal(rec[:], den[:])
            dfa = pp.tile([128, NCOL], FP32, tag="dfa")
            dfb = pp.tile([128, NCOL], FP32, tag="dfb")
            nc.vector.tensor_tensor(out=dfa[:], in0=Abuf[:], in1=rec[:],
                                    op=ALU.mult)

            cand = pp.tile([128, TOPK], FP32, tag="cand")
            zero8 = pp.tile([128, 8], FP32, tag="zero8")
            nc.vector.memset(zero8[:], 0.0)
            rep8 = pp.tile([128, 8], FP32, tag="rep8")
            bufs = [dfa, dfb]
            for j in range(TOPK):
                src, dst = bufs[j % 2], bufs[(j + 1) % 2]
                nc.vector.reduce_max(cand[:, j:j + 1], src[:], axis=AX.X)
                nc.vector.tensor_scalar(out=rep8[:], in0=zero8[:],
                                        scalar1=cand[:, j:j + 1], scalar2=None,
                                        op0=ALU.add)
                nc.vector.match_replace(dst[:], rep8[:], src[:], -1e30)
            nc.sync.dma_start(cand_d.ap(), cand[:])

    nc.compile()
    return nc


def _get_nc():
    global _cached
    if _cached is None:
        _cached = _build()
    return _cached


def kernel(**inputs):
    nc = _get_nc()
    x = np.ascontiguousarray(inputs["x"], dtype=np.float32)
    shared = {k: np.ascontiguousarray(np.asarray(v, dtype=np.float32))
              for k, v in inputs.items() if k != "x"}
    in_maps = []
    for i in range(NCORES):
        m = {"x": np.ascontiguousarray(x[i * NLOC:(i + 1) * NLOC])}
        m.update(shared)
        in_maps.append(m)
    res = bass_utils.run_bass_kernel_spmd(nc, in_maps, core_ids=list(range(NCORES)))
    rs = res.results
    y = np.concatenate([rs[i]["y"] for i in range(NCORES)], axis=0)
    z = np.concatenate([np.ascontiguousarray(rs[i]["z"].T)
                        for i in range(NCORES)], axis=0)
    # global merge (unshard): partial MSE sums + local top-20 candidates
    sse = float(sum(rs[i]["msep"].sum(dtype=np.float64) for i in range(NCORES)))
    mse = sse / (N * D)
    cands = np.concatenate([rs[i]["cand"].ravel() for i in range(NCORES)])
    top = np.sort(cands)[-TOPK:]
    r_loss = np.float32(mse + 0.1 * float(top.sum(dtype=np.float64)))
    return y, z, np.array(r_loss, dtype=np.float32)


# revision 36
# speedup vs baseline: 1.2296x; 1.2296x over previous
"""TRN2 Bass kernel for nn_AE_14413910245386 (autoencoder + cosine-topk loss).

Data-parallel over 8 NeuronCores: x [131072,224] sharded along batch into
8 x [16384,224]; MLP weights + data_input replicated. Each core computes its
y/z shard plus per-partition top-20 cosine candidates and a partial MSE sum;
the host merges candidates (local top-20 -> global top-20) and partial sums
into R_loss during the unshard step.

Per-core layout: feature-major activations (hT [feat, batch]) so the PE
contracts over the partition dim; batch tiles of 512. x is transposed on-chip
via the DMA xbar (bf16); the last layer runs batch-major (lhsT = g2T chunk)
so y lands batch-major in PSUM and needs no transpose back. bd3 is folded
into the last matmul via a ones-row in lhsT / bias-row in rhs.
"""
import numpy as np

import concourse.bacc as bacc
import concourse.tile as tile
import concourse.mybir as mybir
import concourse.bass_utils as bass_utils

N, D, H, Z = 131072, 224, 400, 4
TOPK = 20
NCORES = 8
NLOC = N // NCORES          # 16384 rows per core
BT = 512                    # batch tile
NT = NLOC // BT             # 32 tiles
HCH = [(0, 128), (128, 128), (256, 128), (384, 16)]   # H=400 chunking
DCH = [(0, 128), (128, 96)]                            # D=224 chunking
NCOL = NT * (BT // 128)     # 128 accumulator columns (one per 128-sample chunk)

FP32 = mybir.dt.float32
BF16 = mybir.dt.bfloat16
AF = mybir.ActivationFunctionType
ALU = mybir.AluOpType
AX = mybir.AxisListType

_cached = None


def _build():
    nc = bacc.Bacc("TRN2", target_bir_lowering=False, debug=False,
                   num_devices=NCORES)

    x_d = nc.dram_tensor("x", [NLOC, D], FP32, kind="ExternalInput")
    di_d = nc.dram_tensor("data_input", [D, 1], FP32, kind="ExternalInput")
    w_d = {}
    for name, shape in [("We1", [D, H]), ("be1", [H]), ("We2", [H, H]),
                        ("be2", [H]), ("We3", [H, Z]), ("be3", [Z]),
                        ("Wd1", [Z, H]), ("bd1", [H]), ("Wd2", [H, H]),
                        ("bd2", [H]), ("Wd3", [H, D]), ("bd3", [D])]:
        w_d[name] = nc.dram_tensor(name, shape, FP32, kind="ExternalInput")

    y_d = nc.dram_tensor("y", [NLOC, D], FP32, kind="ExternalOutput")
    # z stored feature-major [Z, NLOC] (contiguous DMA); host transposes back
    z_d = nc.dram_tensor("z", [Z, NLOC], FP32, kind="ExternalOutput")
    cand_d = nc.dram_tensor("cand", [128, TOPK], FP32, kind="ExternalOutput")
    msep_d = nc.dram_tensor("msep", [128, 1], FP32, kind="ExternalOutput")

    with tile.TileContext(nc) as tc:
        with (
            tc.tile_pool(name="wp", bufs=1) as wp,
            tc.tile_pool(name="stg", bufs=2) as stg,
            tc.tile_pool(name="persist", bufs=1) as pp,
            tc.tile_pool(name="xbm", bufs=4) as xbm_p,
            tc.tile_pool(name="xbf", bufs=3) as xbf_p,
            tc.tile_pool(name="xT", bufs=4) as xT_p,
            tc.tile_pool(name="act", bufs=2) as act_p,
            tc.tile_pool(name="ybm", bufs=3) as ybm_p,
            tc.tile_pool(name="scr", bufs=3) as scr_p,
            tc.tile_pool(name="ps", bufs=8, space="PSUM") as ps_p,
        ):
            # ---------------- weight prep (once) ----------------
            def load_lhsT(wname, kchunks, mchunks):
                """W [K, M] -> bf16 lhsT tiles per (kchunk, mchunk)."""
                tiles = {}
                for ki, (k0, kk) in enumerate(kchunks):
                    for mi, (m0, mm) in enumerate(mchunks):
                        s = stg.tile([kk, mm], FP32, tag="wstg")
                        nc.sync.dma_start(s[:], w_d[wname].ap()[k0:k0 + kk, m0:m0 + mm])
                        t = wp.tile([kk, mm], BF16, tag=f"{wname}_{ki}_{mi}")
                        nc.vector.tensor_copy(t[:], s[:])
                        tiles[(ki, mi)] = t
                return tiles

            We1_t = load_lhsT("We1", DCH, HCH)
            We2_t = load_lhsT("We2", HCH, HCH)
            We3_t = load_lhsT("We3", HCH, [(0, Z)])
            Wd1_t = load_lhsT("Wd1", [(0, Z)], HCH)
            Wd2_t = load_lhsT("Wd2", HCH, HCH)

            # Wd3 as rhs tiles [kk, D]; bd3 as its own rhs row (added to the
            # L6 accumulation via a K=1 matmul against a ones-row lhsT)
            Wd3_t = []
            for ki, (k0, kk) in enumerate(HCH):
                s = stg.tile([kk, D], FP32, tag="wstg_d3")
                nc.sync.dma_start(s[:], w_d["Wd3"].ap()[k0:k0 + kk, :])
                t = wp.tile([kk, D], BF16, tag=f"Wd3_{ki}")
                nc.vector.tensor_copy(t[:], s[:])
                Wd3_t.append(t)
            sbd3 = stg.tile([1, D], FP32, tag="wstg_d3b")
            nc.sync.dma_start(sbd3[:], w_d["bd3"].ap())
            bd3_row = wp.tile([1, D], BF16, tag="bd3_row")
            nc.vector.tensor_copy(bd3_row[:], sbd3[:])
            ones_row = wp.tile([1, 128], BF16, tag="ones_row")
            nc.vector.memset(ones_row[:], 1.0)

            # identity [128,128] bf16 for PE-mode transposes
            iph = wp.tile([128, 128], mybir.dt.int32, tag="iph")
            nc.gpsimd.iota(iph[:], pattern=[[1, 128]], base=0,
                           channel_multiplier=0)
            iphf = wp.tile([128, 128], FP32, tag="iphf")
            nc.vector.tensor_copy(iphf[:], iph[:])
            ipp = wp.tile([128, 1], mybir.dt.int32, tag="ipp")
            nc.gpsimd.iota(ipp[:], pattern=[[0, 1]], base=0,
                           channel_multiplier=1)
            ippf = wp.tile([128, 1], FP32, tag="ippf")
            nc.vector.tensor_copy(ippf[:], ipp[:])
            ident = wp.tile([128, 128], BF16, tag="ident")
            nc.vector.tensor_scalar(out=ident[:], in0=iphf[:],
                                    scalar1=ippf[:, 0:1], scalar2=None,
                                    op0=ALU.is_equal)

            def load_bias(bname, chunks):
                out = []
                for mi, (m0, mm) in enumerate(chunks):
                    t = wp.tile([mm, 1], FP32, tag=f"{bname}_{mi}")
                    nc.sync.dma_start(t[:], w_d[bname].ap()[m0:m0 + mm])
                    out.append(t)
                return out

            be1_c = load_bias("be1", HCH)
            be2_c = load_bias("be2", HCH)
            be3_c = load_bias("be3", [(0, Z)])[0]
            bd1_c = load_bias("bd1", HCH)
            bd2_c = load_bias("bd2", HCH)

            # data_input -> dT row, broadcast [128, D]; C = ||d||
            dT = pp.tile([1, D], FP32, tag="dT")
            nc.sync.dma_start(dT[:], di_d.ap().rearrange("d one -> one d"))
            d_bc = pp.tile([128, D], FP32, tag="d_bc")
            nc.gpsimd.partition_broadcast(d_bc[:], dT[:])
            scrd = pp.tile([1, D], FP32, tag="scrd")
            C2 = pp.tile([1, 1], FP32, tag="C2")
            nc.vector.scalar_tensor_tensor(out=scrd[:], in0=dT[:], scalar=1.0,
                                           in1=dT[:], op0=ALU.mult, op1=ALU.mult,
                                           accum_out=C2[:])
            Cv = pp.tile([1, 1], FP32, tag="Cv")
            nc.scalar.activation(Cv[:], C2[:], AF.Sqrt)
            C_bc = pp.tile([128, 1], FP32, tag="C_bc")
            nc.gpsimd.partition_broadcast(C_bc[:], Cv[:])

            # accumulators
            Abuf = pp.tile([128, NCOL], FP32, tag="Abuf")
            B2buf = pp.tile([128, NCOL], FP32, tag="B2buf")
            T2buf = pp.tile([128, NCOL], FP32, tag="T2buf")

            # ------------- main loop: x-prep prefetched 2 tiles ahead ------
            SUP = 2
            loaded = {}
            prep = {}

            def do_load(t):
                s0 = t * BT
                xc, xbf = [], []
                for c in range(4):
                    r0 = s0 + 128 * c
                    xt = xbm_p.tile([128, D], FP32, tag=f"xbm{c}")
                    nc.gpsimd.dma_start(xt[:], x_d.ap()[r0:r0 + 128, :])
                    xc.append(xt)
                    xb = xbf_p.tile([128, D], BF16, tag=f"xbf{c}")
                    nc.vector.tensor_copy(xb[:], xt[:])
                    xbf.append(xb)
                loaded[t] = (xc, xbf)

            def do_transpose(t):
                xc, xbf = loaded.pop(t)
                xT0ps = ps_p.tile([128, BT], BF16, tag="ps")
                xT1ps = ps_p.tile([96, BT], BF16, tag="ps")
                for c in range(4):
                    nc.tensor.transpose(xT0ps[:, 128 * c:128 * (c + 1)],
                                        xbf[c][:, 0:128], ident[:])
                    nc.tensor.transpose(xT1ps[:, 128 * c:128 * (c + 1)],
                                        xbf[c][:, 128:224], ident[:])
                xT0 = xT_p.tile([128, BT], BF16, tag="xT0")
                nc.vector.tensor_copy(xT0[:], xT0ps[:])
                xT1 = xT_p.tile([96, BT], BF16, tag="xT1")
                nc.vector.tensor_copy(xT1[:], xT1ps[:])
                prep[t] = (xc, [xT0, xT1])

            for tp in range(min(SUP, NT)):
                do_load(tp)

            def L1(xTs):
                pss = [ps_p.tile([mm, BT], FP32, tag="ps", name=f"ps_h1_{mi}")
                       for mi, (m0, mm) in enumerate(HCH)]
                for ki, (k0, kk) in enumerate(DCH):
                    for mi in range(4):
                        nc.tensor.matmul(pss[mi][:], We1_t[(ki, mi)][:],
                                         xTs[ki][0:kk, :],
                                         start=(ki == 0), stop=(ki == 1))
                h1 = []
                for mi, (m0, mm) in enumerate(HCH):
                    ht = act_p.tile([mm, BT], BF16, tag=f"h1_{mi}")
                    nc.scalar.activation(ht[:], pss[mi][:], AF.Prelu,
                                         bias=be1_c[mi][:], alpha=0.1)
                    h1.append(ht)
                return h1

            def LMID(h_in, W_t, b_c, name):
                # K-outer emission: all 4 PSUM groups allocated up front,
                # K-chunks interleaved -> slot-wait stalls concentrate once
                pss = [ps_p.tile([mm, BT], FP32, tag="ps", name=f"ps_{name}_{mi}")
                       for mi, (m0, mm) in enumerate(HCH)]
                for ki in range(4):
                    for mi in range(4):
                        nc.tensor.matmul(pss[mi][:], W_t[(ki, mi)][:],
                                         h_in[ki][:],
                                         start=(ki == 0), stop=(ki == 3))
                out = []
                for mi, (m0, mm) in enumerate(HCH):
                    ht = act_p.tile([mm, BT], BF16, tag=f"{name}_{mi}")
                    nc.scalar.activation(ht[:], pss[mi][:], AF.Prelu,
                                         bias=b_c[mi][:], alpha=0.1)
                    out.append(ht)
                return out

            def L3(h2, s0):
                ps = ps_p.tile([Z, BT], FP32, tag="ps")
                for ki in range(4):
                    nc.tensor.matmul(ps[:], We3_t[(ki, 0)][:], h2[ki][:],
                                     start=(ki == 0), stop=(ki == 3))
                zbf = act_p.tile([Z, BT], BF16, tag="zbf")
                nc.scalar.activation(zbf[:], ps[:], AF.Identity,
                                     bias=be3_c[:])
                z_f = act_p.tile([Z, BT], FP32, tag="z_f")
                nc.vector.tensor_scalar(out=z_f[:], in0=ps[:],
                                        scalar1=be3_c[:], scalar2=None,
                                        op0=ALU.add)
                nc.sync.dma_start(z_d.ap()[:, s0:s0 + BT], z_f[:])
                return zbf

            def L4(zbf):
                g1 = []
                for mi, (m0, mm) in enumerate(HCH):
                    ps = ps_p.tile([mm, BT], FP32, tag="ps")
                    nc.tensor.matmul(ps[:], Wd1_t[(0, mi)][:], zbf[:],
                                     start=True, stop=True)
                    ht = act_p.tile([mm, BT], BF16, tag=f"g1_{mi}")
                    nc.scalar.activation(ht[:], ps[:], AF.Prelu,
                                         bias=bd1_c[mi][:], alpha=0.1)
                    g1.append(ht)
                return g1

            def L6(g2, xc, t, s0):
                pss = [ps_p.tile([128, D], FP32, tag="ps", name=f"ps_y_{s}")
                       for s in range(4)]
                for ki in range(4):
                    for s in range(4):
                        nc.tensor.matmul(pss[s][:],
                                         g2[ki][:, 128 * s:128 * (s + 1)],
                                         Wd3_t[ki][:],
                                         start=(ki == 0), stop=False)
                for s in range(4):
                    nc.tensor.matmul(pss[s][:], ones_row[:], bd3_row[:],
                                     start=False, stop=True)
                for s in range(4):
                    ps = pss[s]
                    yt = ybm_p.tile([128, D], FP32, tag=f"ybm{s}")
                    nc.scalar.activation(yt[:], ps[:], AF.Tanh)
                    nc.sync.dma_start(y_d.ap()[s0 + 128 * s:s0 + 128 * (s + 1), :],
                                      yt[:])
                    # fused row-reductions into accumulator column j
                    j = 4 * t + s
                    sA = scr_p.tile([128, D], FP32, tag="sA")
                    nc.vector.scalar_tensor_tensor(
                        out=sA[:], in0=yt[:], scalar=1.0, in1=d_bc[:],
                        op0=ALU.mult, op1=ALU.mult,
                        accum_out=Abuf[:, j:j + 1])
                    sB = scr_p.tile([128, D], FP32, tag="sB")
                    nc.vector.scalar_tensor_tensor(
                        out=sB[:], in0=yt[:], scalar=1.0, in1=yt[:],
                        op0=ALU.mult, op1=ALU.mult,
                        accum_out=B2buf[:, j:j + 1])
                    td = scr_p.tile([128, D], FP32, tag="td")
                    nc.vector.tensor_tensor(out=td[:], in0=yt[:], in1=xc[s][:],
                                            op=ALU.subtract)
                    sC = scr_p.tile([128, D], FP32, tag="sC")
                    nc.vector.scalar_tensor_tensor(
                        out=sC[:], in0=td[:], scalar=1.0, in1=td[:],
                        op0=ALU.mult, op1=ALU.mult,
                        accum_out=T2buf[:, j:j + 1])

            do_transpose(0)
            do_transpose(1)
            for t0 in range(0, NT, 2):
                t1 = t0 + 1
                (xc0, xTs0), (xc1, xTs1) = prep.pop(t0), prep.pop(t1)
                h1_0 = L1(xTs0)
                h1_1 = L1(xTs1)
                h2_0 = LMID(h1_0, We2_t, be2_c, "h2")
                h2_1 = LMID(h1_1, We2_t, be2_c, "h2")
                zbf0 = L3(h2_0, t0 * BT)
                zbf1 = L3(h2_1, t1 * BT)
                if t0 + 2 < NT:
                    do_load(t0 + 2)
                    do_transpose(t0 + 2)
                if t1 + 2 < NT:
                    do_load(t1 + 2)
                    do_transpose(t1 + 2)
                g1_0 = L4(zbf0)
                g1_1 = L4(zbf1)
                g2_0 = LMID(g1_0, Wd2_t, bd2_c, "g2")
                g2_1 = LMID(g1_1, Wd2_t, bd2_c, "g2")
                L6(g2_0, xc0, t0, t0 * BT)
                L6(g2_1, xc1, t1, t1 * BT)

            # ---------------- end phase ----------------
            msep_sb = pp.tile([128, 1], FP32, tag="msep_sb")
            nc.vector.reduce_sum(msep_sb[:], T2buf[:], axis=AX.X)
            nc.sync.dma_start(msep_d.ap(), msep_sb[:])

            Bsq = pp.tile([128, NCOL], FP32, tag="Bsq")
            nc.scalar.activation(Bsq[:], B2buf[:], AF.Sqrt)
            den = pp.tile([128, NCOL], FP32, tag="den")
            nc.vector.tensor_scalar(out=den[:], in0=Bsq[:], scalar1=C_bc[:, 0:1],
                                    scalar2=1e-5, op0=ALU.mult, op1=ALU.add)
            rec = pp.tile([128, NCOL], FP32, tag="rec")
            nc.vector.reciprocal(rec[:], den[:])
            dfa = pp.tile([128, NCOL], FP32, tag="dfa")
            dfb = pp.tile([128, NCOL], FP32, tag="dfb")
            nc.vector.tensor_tensor(out=dfa[:], in0=Abuf[:], in1=rec[:],
                                    op=ALU.mult)

            cand = pp.tile([128, TOPK], FP32, tag="cand")
            zero8 = pp.tile([128, 8], FP32, tag="zero8")
            nc.vector.memset(zero8[:], 0.0)
            rep8 = pp.tile([128, 8], FP32, tag="rep8")
            bufs = [dfa, dfb]
            for j in range(TOPK):
                src, dst = bufs[j % 2], bufs[(j + 1) % 2]
                nc.vector.reduce_max(cand[:, j:j + 1], src[:], axis=AX.X)
                nc.vector.tensor_scalar(out=rep8[:], in0=zero8[:],
                                        scalar1=cand[:, j:j + 1], scalar2=None,
                                        op0=ALU.add)
                nc.vector.match_replace(dst[:], rep8[:], src[:], -1e30)
            nc.sync.dma_start(cand_d.ap(), cand[:])

    nc.compile()
    return nc


def _get_nc():
    global _cached
    if _cached is None:
        _cached = _build()
    return _cached


def kernel(**inputs):
    nc = _get_nc()
    x = np.ascontiguousarray(inputs["x"], dtype=np.float32)
    shared = {k: np.ascontiguousarray(np.asarray(v, dtype=np.float32))
              for k, v in inputs.items() if k != "x"}
    in_maps = []
    for i in range(NCORES):
        m = {"x": np.ascontiguousarray(x[i * NLOC:(i + 1) * NLOC])}
        m.update(shared)
        in_maps.append(m)
    res = bass_utils.run_bass_kernel_spmd(nc, in_maps, core_ids=list(range(NCORES)))
    rs = res.results
    y = np.concatenate([rs[i]["y"] for i in range(NCORES)], axis=0)
    z = np.concatenate([np.ascontiguousarray(rs[i]["z"].T)
                        for i in range(NCORES)], axis=0)
    # global merge (unshard): partial MSE sums + local top-20 candidates
    sse = float(sum(rs[i]["msep"].sum(dtype=np.float64) for i in range(NCORES)))
    mse = sse / (N * D)
    cands = np.concatenate([rs[i]["cand"].ravel() for i in range(NCORES)])
    top = np.sort(cands)[-TOPK:]
    r_loss = np.float32(mse + 0.1 * float(top.sum(dtype=np.float64)))
    return y, z, np.array(r_loss, dtype=np.float32)
